# revision 1
# baseline (speedup 1.0000x reference)
"""nn_AttnBlock Trainium2 kernel (Bass/Tile), data-parallel over batch on 8 cores.

Contract: kernel(**inputs) takes the FULL unsharded inputs (as produced by
setup_inputs(): x [16,512,32,32] f32, gn_gamma/gn_beta [512], Wq/bq/Wk/bk/
Wv/bv/Wp/bp) and returns the FULL output [16,512,32,32] f32.

Strategy (per core = 2 samples; no cross-core communication):
  h   = GroupNorm(x)*gamma+beta              [c=512, t=1024] bf16, c on 4x128 partitions
  g   = (Wq^T Wk / sqrt(c))^T-applied: g = wm^T h        [c, i]
  vpT = (Wp Wv h)^T  (fused v+proj)                      [j, o']
  S^T = h^T g  -> E = exp(S^T)  (no max-subtraction; scores are O(1))
  Z   = ones^T E   (PE column sums), zrb = broadcast(1/Z)
  p   = vpT^T E
  y   = p * zrb + (x + (Wp bv + bp))
The q/k fusion is exact when bq == bk == 0 (the spec fill); otherwise an
unfused variant with explicit q/k biases is built instead.
"""
import math
import sys
from contextlib import ExitStack

if "/opt/trn_rl_repo" not in sys.path:
    sys.path.insert(0, "/opt/trn_rl_repo")

import numpy as np
import ml_dtypes

import concourse.bass as bass
import concourse.tile as tile
from concourse import mybir
from concourse.bass_utils import run_bass_kernel_spmd

F32 = mybir.dt.float32
BF16 = mybir.dt.bfloat16

B = 16
C = 512
H = 32
W = 32
HW = H * W
NCHUNK = 4          # C / 128 partition chunks
NJT = 8             # HW / 128 key tiles
NNI = 2             # HW / 512 free-dim chunks
G = 16              # groups
EPS = 1e-6
N_CORES = 8
SPC = B // N_CORES  # samples per core


def _split_multiwait_drains(nc, max_waits=1):
    """walrus in this container rejects instructions carrying >1 sem waits
    ('Too many sync wait commands'); split extras into preceding single-wait
    Drain nops on the same engine."""
    f = nc.m.functions[0]
    ctr = 0
    for blk in f.blocks:
        insts = blk.instructions
        i = 0
        while i < len(insts):
            inst = insts[i]
            si = inst.sync_info
            waits = list(si.on_wait) if si and si.on_wait else []
            if len(waits) > max_waits:
                si.on_wait = waits[:max_waits]
                inst.sync_info = si
                for j, w in enumerate(waits[max_waits:]):
                    d = mybir.InstDrain(name=f"waitsplit_{ctr}", engine=inst.engine)
                    ctr += 1
                    d.sync_info = mybir.SyncInfo(on_wait=[w], on_update=[])
                    insts.insert(i + j, d)
                i += len(waits) - max_waits
            i += 1


def build(reps=1, fused=True):
    nc = bass.Bass("TRN2", target_bir_lowering=False, debug=False, num_devices=N_CORES)

    x_ext = nc.dram_tensor("x", [SPC, C, HW], F32, kind="ExternalInput").ap()
    wm_ext = nc.dram_tensor("wm", [C, C], BF16, kind="ExternalInput").ap()
    wvp_ext = nc.dram_tensor("wvp", [C, C], BF16, kind="ExternalInput").ap()
    if not fused:
        wk_ext = nc.dram_tensor("wk", [C, C], BF16, kind="ExternalInput").ap()
    # rows: gamma, beta, bpp, bq(scaled), bk
    vecs_ext = nc.dram_tensor("vecs", [5, C], F32, kind="ExternalInput").ap()
    mfw_ext = nc.dram_tensor("mask_fwd", [128, NCHUNK, G], F32, kind="ExternalInput").ap()
    mbw_ext = nc.dram_tensor("mask_bwd", [G, NCHUNK, 128], F32, kind="ExternalInput").ap()
    y_ext = nc.dram_tensor("y", [SPC, C, HW], F32, kind="ExternalOutput").ap()

    with tile.TileContext(nc) as tc, ExitStack() as ctx:
        pw = ctx.enter_context(tc.tile_pool(name="pw", bufs=1))
        px = ctx.enter_context(tc.tile_pool(name="px", bufs=2))
        ph = ctx.enter_context(tc.tile_pool(name="ph", bufs=2))
        pq = ctx.enter_context(tc.tile_pool(name="pq", bufs=2))
        pk = ctx.enter_context(tc.tile_pool(name="pk", bufs=2))
        pvt = ctx.enter_context(tc.tile_pool(name="pvt", bufs=2))
        pe = ctx.enter_context(tc.tile_pool(name="pe", bufs=2 if fused else 1))
        py = ctx.enter_context(tc.tile_pool(name="py", bufs=2))
        ptail = ctx.enter_context(tc.tile_pool(name="ptail", bufs=2))
        pzrb = ctx.enter_context(tc.tile_pool(name="pzrb", bufs=2))
        pzr = ctx.enter_context(tc.tile_pool(name="pzr", bufs=2))
        pg = ctx.enter_context(tc.tile_pool(name="pg", bufs=4))
        pp_big = ctx.enter_context(tc.tile_pool(name="pp_big", bufs=5, space="PSUM"))
        pp_z = ctx.enter_context(tc.tile_pool(name="pp_z", bufs=1, space="PSUM"))
        pp_small = ctx.enter_context(tc.tile_pool(name="pp_small", bufs=1, space="PSUM"))

        wm_sb = pw.tile([128, NCHUNK, C], BF16, tag="wm")
        nc.gpsimd.dma_start(out=wm_sb, in_=wm_ext.rearrange("(ci p) o -> p ci o", p=128))
        wvp_sb = pw.tile([128, NCHUNK, C], BF16, tag="wvp")
        nc.gpsimd.dma_start(out=wvp_sb, in_=wvp_ext.rearrange("(ci p) o -> p ci o", p=128))
        if not fused:
            wk_sb = pw.tile([128, NCHUNK, C], BF16, tag="wk")
            nc.gpsimd.dma_start(out=wk_sb, in_=wk_ext.rearrange("(ci p) o -> p ci o", p=128))
        vecs_sb = pw.tile([128, 5, NCHUNK], F32, tag="vecs")
        nc.gpsimd.dma_start(out=vecs_sb, in_=vecs_ext.rearrange("v (ci p) -> p v ci", p=128))
        gamma_sb = vecs_sb[:, 0, :]
        beta_sb = vecs_sb[:, 1, :]
        bpp_sb = vecs_sb[:, 2, :]
        bq_sb = vecs_sb[:, 3, :]
        bk_sb = vecs_sb[:, 4, :]
        mfw_sb = pw.tile([128, NCHUNK, G], F32, tag="mfw")
        nc.gpsimd.dma_start(out=mfw_sb, in_=mfw_ext)
        mbw_sb = pw.tile([G, NCHUNK, 128], F32, tag="mbw")
        nc.gpsimd.dma_start(out=mbw_sb, in_=mbw_ext)
        ones_sb = pw.tile([128, 1], BF16, tag="ones")
        nc.vector.memset(ones_sb, 1.0)
        onesf_sb = pw.tile([1, 128], F32, tag="onesf")
        nc.vector.memset(onesf_sb, 1.0)
        eps_sb = pw.tile([128, 1], F32, tag="eps")
        nc.vector.memset(eps_sb, EPS)

        for rep in range(reps):
            for s in range(SPC):
                x_t = px.tile([128, NCHUNK, HW], F32, tag="x")
                nc.sync.dma_start(
                    out=x_t, in_=x_ext[s].rearrange("(ci p) t -> p ci t", p=128)
                )

                # ---- GroupNorm stats: per-partition bn_stats, then group
                # aggregation and per-channel broadcast via tiny mask matmuls.
                stat3 = pg.tile([128, NCHUNK, 3], F32, tag="stat3")
                for ci in range(NCHUNK):
                    st6 = pg.tile([128, 2, 6], F32, tag="st6")
                    for half in range(2):
                        nc.vector.bn_stats(
                            out=st6[:, half, :],
                            in_=x_t[:, ci, half * 512:(half + 1) * 512],
                        )
                    nc.vector.bn_aggr(out=stat3[:, ci, 0:2], in_=st6)
                    nc.vector.tensor_mul(
                        stat3[:, ci, 2:3], stat3[:, ci, 0:1], stat3[:, ci, 0:1]
                    )
                psum_g = pp_small.tile([G, 3], F32, tag="pssm")
                for ci in range(NCHUNK):
                    nc.tensor.matmul(
                        psum_g, mfw_sb[:, ci, :], stat3[:, ci, :],
                        start=(ci == 0), stop=(ci == NCHUNK - 1),
                    )
                gsb = pg.tile([G, 3], F32, tag="gsb")
                nc.scalar.activation(
                    out=gsb, in_=psum_g, func=mybir.ActivationFunctionType.Copy,
                )
                gs = pg.tile([G, 2], F32, tag="gs")
                t0 = pg.tile([G, 1], F32, tag="t0")
                nc.vector.tensor_mul(t0, gsb[:, 0:1], gsb[:, 0:1])
                nc.vector.tensor_add(gs[:, 1:2], gsb[:, 1:2], gsb[:, 2:3])
                nc.vector.tensor_sub(gs[:, 1:2], gs[:, 1:2], t0)
                # rsqrt(v+eps) = exp(-0.5*ln(v+eps)); Ln/Exp share an ACT table set
                nc.scalar.activation(
                    out=gs[:, 1:2], in_=gs[:, 1:2],
                    func=mybir.ActivationFunctionType.Ln,
                    bias=eps_sb[0:G], scale=1.0,
                )
                nc.scalar.activation(
                    out=gs[:, 1:2], in_=gs[:, 1:2],
                    func=mybir.ActivationFunctionType.Exp,
                    bias=0.0, scale=-0.5,
                )
                nc.vector.tensor_copy(out=gs[:, 0:1], in_=gsb[:, 0:1])

                # ---- normalize + affine -> h (bf16) ----
                h_t = ph.tile([128, NCHUNK, HW], BF16, tag="h")
                for ci in range(NCHUNK):
                    psum_bc = pp_small.tile([128, 2], F32, tag="pssm")
                    nc.tensor.matmul(psum_bc, mbw_sb[:, ci, :], gs, start=True, stop=True)
                    A = pg.tile([128, 1], F32, tag="A")
                    Bt = pg.tile([128, 1], F32, tag="B")
                    nc.vector.tensor_mul(A, psum_bc[:, 1:2], gamma_sb[:, ci:ci + 1])
                    nc.vector.tensor_mul(Bt, psum_bc[:, 0:1], A)
                    nc.vector.tensor_sub(Bt, beta_sb[:, ci:ci + 1], Bt)
                    nc.vector.tensor_scalar(
                        out=h_t[:, ci, :], in0=x_t[:, ci, :],
                        scalar1=A, scalar2=Bt,
                        op0=mybir.AluOpType.mult, op1=mybir.AluOpType.add,
                    )

                # ---- g (fused: wm^T h == q with k-side folded) ----
                g_t = pq.tile([128, NCHUNK, HW], BF16, tag="g")
                for mo in range(NCHUNK):
                    for ni in range(NNI):
                        ps = pp_big.tile([128, 512], F32, tag="ps")
                        for ci in range(NCHUNK):
                            nc.tensor.matmul(
                                ps,
                                wm_sb[:, ci, mo * 128:(mo + 1) * 128],
                                h_t[:, ci, ni * 512:(ni + 1) * 512],
                                start=(ci == 0), stop=(ci == NCHUNK - 1),
                            )
                        if fused:
                            nc.scalar.activation(
                                out=g_t[:, mo, ni * 512:(ni + 1) * 512], in_=ps,
                                func=mybir.ActivationFunctionType.Copy,
                            )
                        else:
                            nc.scalar.activation(
                                out=g_t[:, mo, ni * 512:(ni + 1) * 512], in_=ps,
                                func=mybir.ActivationFunctionType.Identity,
                                bias=bq_sb[:, mo:mo + 1], scale=1.0,
                            )

                if not fused:
                    k_t = pk.tile([128, NCHUNK, HW], BF16, tag="k")
                    for mo in range(NCHUNK):
                        for ni in range(NNI):
                            ps = pp_big.tile([128, 512], F32, tag="ps")
                            for ci in range(NCHUNK):
                                nc.tensor.matmul(
                                    ps,
                                    wk_sb[:, ci, mo * 128:(mo + 1) * 128],
                                    h_t[:, ci, ni * 512:(ni + 1) * 512],
                                    start=(ci == 0), stop=(ci == NCHUNK - 1),
                                )
                            nc.scalar.activation(
                                out=k_t[:, mo, ni * 512:(ni + 1) * 512], in_=ps,
                                func=mybir.ActivationFunctionType.Identity,
                                bias=bk_sb[:, mo:mo + 1], scale=1.0,
                            )
                    s_lhs = k_t
                else:
                    s_lhs = h_t

                # ---- vpT = (Wp Wv h)^T : [j, o'] ----
                vpT_t = pvt.tile([128, NJT, C], BF16, tag="vpT")
                for jo in range(NJT):
                    ps = pp_big.tile([128, 512], F32, tag="ps")
                    for ci in range(NCHUNK):
                        nc.tensor.matmul(
                            ps,
                            h_t[:, ci, jo * 128:(jo + 1) * 128],
                            wvp_sb[:, ci, :],
                            start=(ci == 0), stop=(ci == NCHUNK - 1),
                        )
                    nc.scalar.activation(
                        out=vpT_t[:, jo, :], in_=ps,
                        func=mybir.ActivationFunctionType.Copy,
                    )

                # ---- S^T = s_lhs^T g, E = exp, Z column sums (PE) ----
                e_t = pe.tile([128, NJT, HW], BF16, tag="e")
                psz = pp_z.tile([64, 512], F32, tag="psz")
                for jo in range(NJT):
                    for ni in range(NNI):
                        ps = pp_big.tile([128, 512], F32, tag="ps")
                        for ci in range(NCHUNK):
                            nc.tensor.matmul(
                                ps,
                                s_lhs[:, ci, jo * 128:(jo + 1) * 128],
                                g_t[:, ci, ni * 512:(ni + 1) * 512],
                                start=(ci == 0), stop=(ci == NCHUNK - 1),
                            )
                        nc.scalar.activation(
                            out=e_t[:, jo, ni * 512:(ni + 1) * 512], in_=ps,
                            func=mybir.ActivationFunctionType.Exp,
                        )
                        nc.tensor.matmul(
                            psz[ni * 32:ni * 32 + 1, :], ones_sb,
                            e_t[:, jo, ni * 512:(ni + 1) * 512],
                            start=(jo == 0), stop=(jo == NJT - 1),
                        )

                # ---- 1/Z, broadcast across partitions via rank-1 matmul ----
                zr = pzr.tile([1, HW], F32, tag="zr", name=f"zr_{rep}_{s}")
                for ni in range(NNI):
                    nc.vector.reciprocal(
                        out=zr[:, ni * 512:(ni + 1) * 512],
                        in_=psz[ni * 32:ni * 32 + 1, :]
                    )
                zrb = pzrb.tile([128, HW], F32, tag="zrb", name=f"zrb_{rep}_{s}")
                for ni in range(NNI):
                    ps = pp_big.tile([128, 512], F32, tag="ps", name=f"zb_{rep}_{s}_{ni}")
                    nc.tensor.matmul(
                        ps, onesf_sb, zr[:, ni * 512:(ni + 1) * 512],
                        start=True, stop=True,
                    )
                    nc.scalar.activation(
                        out=zrb[:, ni * 512:(ni + 1) * 512], in_=ps,
                        func=mybir.ActivationFunctionType.Copy,
                    )

                # ---- p = vpT^T E; tail y = p*zr + (x + bpp) ----
                for mo in range(NCHUNK):
                    y_t = py.tile([128, HW], F32, tag="y")
                    for ni in range(NNI):
                        ps = pp_big.tile([128, 512], F32, tag="ps")
                        for jo in range(NJT):
                            nc.tensor.matmul(
                                ps,
                                vpT_t[:, jo, mo * 128:(mo + 1) * 128],
                                e_t[:, jo, ni * 512:(ni + 1) * 512],
                                start=(jo == 0), stop=(jo == NJT - 1),
                            )
                        tmp = ptail.tile([128, 512], F32, tag="tmp")
                        nc.vector.tensor_mul(tmp, ps, zrb[:, ni * 512:(ni + 1) * 512])
                        nc.vector.scalar_tensor_tensor(
                            out=y_t[:, ni * 512:(ni + 1) * 512],
                            in0=x_t[:, mo, ni * 512:(ni + 1) * 512],
                            scalar=bpp_sb[:, mo:mo + 1],
                            in1=tmp,
                            op0=mybir.AluOpType.add, op1=mybir.AluOpType.add,
                        )
                    nc.gpsimd.dma_start(
                        out=y_ext[s, mo * 128:(mo + 1) * 128, :], in_=y_t
                    )

    _split_multiwait_drains(nc)
    return nc


def make_host_inputs(x, gn_gamma, gn_beta, Wq, bq, Wk, bk, Wv, bv, Wp, bp, fused):
    scale = 1.0 / math.sqrt(C)
    wvp_l = ((Wp.astype(np.float64) @ Wv.astype(np.float64)).T).astype(ml_dtypes.bfloat16)
    bpp = (Wp.astype(np.float64) @ bv.astype(np.float64) + bp).astype(np.float32)
    if fused:
        # wm = Wq^T Wk / sqrt(c): S^T = (h^T wm^T) ... exact when bq=bk=0
        wm_l = (Wq.T.astype(np.float64) @ Wk.astype(np.float64) * scale).astype(ml_dtypes.bfloat16)
    else:
        wm_l = np.ascontiguousarray(Wq.T * scale).astype(ml_dtypes.bfloat16)
    wk_l = np.ascontiguousarray(Wk.T).astype(ml_dtypes.bfloat16)
    vecs = np.stack([
        gn_gamma, gn_beta, bpp, bq * scale, bk
    ]).astype(np.float32)

    grp = np.arange(C) // (C // G)
    mfw = np.zeros((128, NCHUNK, G), np.float32)
    mbw = np.zeros((G, NCHUNK, 128), np.float32)
    for ci in range(NCHUNK):
        for p in range(128):
            g = grp[ci * 128 + p]
            mfw[p, ci, g] = 1.0 / (C // G)
            mbw[g, ci, p] = 1.0

    xr = np.ascontiguousarray(x.reshape(B, C, HW)).astype(np.float32)
    in_maps = []
    for i in range(N_CORES):
        m = {
            "x": xr[i * SPC:(i + 1) * SPC],
            "wm": wm_l, "wvp": wvp_l,
            "vecs": vecs, "mask_fwd": mfw, "mask_bwd": mbw,
        }
        if not fused:
            m["wk"] = wk_l
        in_maps.append(m)
    return in_maps


_nc_cache = {}


def kernel(x, gn_gamma, gn_beta, Wq, bq, Wk, bk, Wv, bv, Wp, bp):
    x = np.asarray(x, dtype=np.float32)
    args = {k: np.asarray(v, dtype=np.float32) for k, v in dict(
        gn_gamma=gn_gamma, gn_beta=gn_beta, Wq=Wq, bq=bq, Wk=Wk, bk=bk,
        Wv=Wv, bv=bv, Wp=Wp, bp=bp).items()}
    b, c, h, w = x.shape
    assert (b, c, h * w) == (B, C, HW), f"unexpected shape {x.shape}"

    # q/k fusion is exact only for zero q/k biases (the spec fill)
    fused = (np.abs(args["bq"]).max() == 0.0 and np.abs(args["bk"]).max() == 0.0)

    if fused not in _nc_cache:
        _nc_cache[fused] = build(reps=1, fused=fused)
    nc = _nc_cache[fused]

    in_maps = make_host_inputs(x, fused=fused, **args)
    res = run_bass_kernel_spmd(nc, in_maps, list(range(N_CORES)))
    y = np.concatenate([r["y"] for r in res.results], axis=0)
    return y.reshape(b, c, h, w).astype(np.float32)


if __name__ == "__main__":
    rng = np.random.default_rng(0)
    scale = 1.0 / math.sqrt(C)
    demo = dict(
        x=rng.standard_normal((B, C, H, W), dtype=np.float32),
        gn_gamma=np.ones(C, np.float32), gn_beta=np.zeros(C, np.float32),
        Wq=(rng.standard_normal((C, C)) * scale).astype(np.float32),
        bq=np.zeros(C, np.float32),
        Wk=(rng.standard_normal((C, C)) * scale).astype(np.float32),
        bk=np.zeros(C, np.float32),
        Wv=(rng.standard_normal((C, C)) * scale).astype(np.float32),
        bv=np.zeros(C, np.float32),
        Wp=(rng.standard_normal((C, C)) * scale).astype(np.float32),
        bp=np.zeros(C, np.float32),
    )
    out = kernel(**demo)
    print("kernel output:", out.shape, out.dtype, float(np.abs(out).max()))


# revision 2
# speedup vs baseline: 1.0355x; 1.0355x over previous
"""nn_AttnBlock Trainium2 kernel (Bass/Tile), data-parallel over batch on 8 cores.

Contract: kernel(**inputs) takes the FULL unsharded inputs (as produced by
setup_inputs(): x [16,512,32,32] f32, gn_gamma/gn_beta [512], Wq/bq/Wk/bk/
Wv/bv/Wp/bp) and returns the FULL output [16,512,32,32] f32.

Strategy (per core = 2 samples; no cross-core communication):
  h   = GroupNorm(x)*gamma+beta              [c=512, t=1024] bf16, c on 4x128 partitions
  g   = (Wq^T Wk / sqrt(c))^T-applied: g = wm^T h        [c, i]
  vpT = (Wp Wv h)^T  (fused v+proj)                      [j, o']
  S^T = h^T g  -> E = exp(S^T)  (no max-subtraction; scores are O(1))
  Z   = ones^T E   (PE column sums), zrb = broadcast(1/Z)
  p   = vpT^T E
  y   = p * zrb + (x + (Wp bv + bp))
The q/k fusion is exact when bq == bk == 0 (the spec fill); otherwise an
unfused variant with explicit q/k biases is built instead.
"""
import math
import sys
from contextlib import ExitStack

if "/opt/trn_rl_repo" not in sys.path:
    sys.path.insert(0, "/opt/trn_rl_repo")

import numpy as np
import ml_dtypes

import concourse.bass as bass
import concourse.tile as tile
from concourse import mybir
from concourse.bass_utils import run_bass_kernel_spmd

F32 = mybir.dt.float32
BF16 = mybir.dt.bfloat16

B = 16
C = 512
H = 32
W = 32
HW = H * W
NCHUNK = 4          # C / 128 partition chunks
NJT = 8             # HW / 128 key tiles
NNI = 2             # HW / 512 free-dim chunks
G = 16              # groups
EPS = 1e-6
N_CORES = 8
SPC = B // N_CORES  # samples per core


def _split_multiwait_drains(nc, max_waits=1):
    """walrus in this container rejects instructions carrying >1 sem waits
    ('Too many sync wait commands'); split extras into preceding single-wait
    Drain nops on the same engine."""
    f = nc.m.functions[0]
    ctr = 0
    for blk in f.blocks:
        insts = blk.instructions
        i = 0
        while i < len(insts):
            inst = insts[i]
            si = inst.sync_info
            waits = list(si.on_wait) if si and si.on_wait else []
            if len(waits) > max_waits:
                si.on_wait = waits[:max_waits]
                inst.sync_info = si
                for j, w in enumerate(waits[max_waits:]):
                    d = mybir.InstDrain(name=f"waitsplit_{ctr}", engine=inst.engine)
                    ctr += 1
                    d.sync_info = mybir.SyncInfo(on_wait=[w], on_update=[])
                    insts.insert(i + j, d)
                i += len(waits) - max_waits
            i += 1


def build(reps=1, fused=True):
    nc = bass.Bass("TRN2", target_bir_lowering=False, debug=False, num_devices=N_CORES)

    x_ext = nc.dram_tensor("x", [SPC, C, HW], F32, kind="ExternalInput").ap()
    wm_ext = nc.dram_tensor("wm", [C, C], BF16, kind="ExternalInput").ap()
    wvp_ext = nc.dram_tensor("wvp", [C, C], BF16, kind="ExternalInput").ap()
    if not fused:
        wk_ext = nc.dram_tensor("wk", [C, C], BF16, kind="ExternalInput").ap()
    # rows: gamma, beta, bpp, bq(scaled), bk
    vecs_ext = nc.dram_tensor("vecs", [5, C], F32, kind="ExternalInput").ap()
    mfw_ext = nc.dram_tensor("mask_fwd", [128, NCHUNK, G], F32, kind="ExternalInput").ap()
    mbw_ext = nc.dram_tensor("mask_bwd", [G, NCHUNK, 128], F32, kind="ExternalInput").ap()
    y_ext = nc.dram_tensor("y", [SPC, C, HW], F32, kind="ExternalOutput").ap()

    with tile.TileContext(nc) as tc, ExitStack() as ctx:
        pw = ctx.enter_context(tc.tile_pool(name="pw", bufs=1))
        px = ctx.enter_context(tc.tile_pool(name="px", bufs=2))
        ph = ctx.enter_context(tc.tile_pool(name="ph", bufs=2))
        pq = ctx.enter_context(tc.tile_pool(name="pq", bufs=2))
        pk = ctx.enter_context(tc.tile_pool(name="pk", bufs=2))
        pvt = ctx.enter_context(tc.tile_pool(name="pvt", bufs=2))
        pe = ctx.enter_context(tc.tile_pool(name="pe", bufs=2 if fused else 1))
        py = ctx.enter_context(tc.tile_pool(name="py", bufs=2))
        ptail = ctx.enter_context(tc.tile_pool(name="ptail", bufs=2))
        pzrb = ctx.enter_context(tc.tile_pool(name="pzrb", bufs=2))
        pzr = ctx.enter_context(tc.tile_pool(name="pzr", bufs=2))
        pg = ctx.enter_context(tc.tile_pool(name="pg", bufs=4))
        pp_big = ctx.enter_context(tc.tile_pool(name="pp_big", bufs=5, space="PSUM"))
        pp_z = ctx.enter_context(tc.tile_pool(name="pp_z", bufs=1, space="PSUM"))
        pp_small = ctx.enter_context(tc.tile_pool(name="pp_small", bufs=1, space="PSUM"))

        wm_sb = pw.tile([128, NCHUNK, C], BF16, tag="wm")
        nc.gpsimd.dma_start(out=wm_sb, in_=wm_ext.rearrange("(ci p) o -> p ci o", p=128))
        wvp_sb = pw.tile([128, NCHUNK, C], BF16, tag="wvp")
        nc.gpsimd.dma_start(out=wvp_sb, in_=wvp_ext.rearrange("(ci p) o -> p ci o", p=128))
        if not fused:
            wk_sb = pw.tile([128, NCHUNK, C], BF16, tag="wk")
            nc.gpsimd.dma_start(out=wk_sb, in_=wk_ext.rearrange("(ci p) o -> p ci o", p=128))
        vecs_sb = pw.tile([128, 5, NCHUNK], F32, tag="vecs")
        nc.gpsimd.dma_start(out=vecs_sb, in_=vecs_ext.rearrange("v (ci p) -> p v ci", p=128))
        gamma_sb = vecs_sb[:, 0, :]
        beta_sb = vecs_sb[:, 1, :]
        bpp_sb = vecs_sb[:, 2, :]
        bq_sb = vecs_sb[:, 3, :]
        bk_sb = vecs_sb[:, 4, :]
        mfw_sb = pw.tile([128, NCHUNK, G], F32, tag="mfw")
        nc.gpsimd.dma_start(out=mfw_sb, in_=mfw_ext)
        mbw_sb = pw.tile([G, NCHUNK, 128], F32, tag="mbw")
        nc.gpsimd.dma_start(out=mbw_sb, in_=mbw_ext)
        ones_sb = pw.tile([128, 1], BF16, tag="ones")
        nc.vector.memset(ones_sb, 1.0)
        onesf_sb = pw.tile([1, 128], F32, tag="onesf")
        nc.vector.memset(onesf_sb, 1.0)
        eps_sb = pw.tile([128, 1], F32, tag="eps")
        nc.vector.memset(eps_sb, EPS)

        for rep in range(reps):
            for s in range(SPC):
                x_t = px.tile([128, NCHUNK, HW], F32, tag="x")
                nc.sync.dma_start(
                    out=x_t, in_=x_ext[s].rearrange("(ci p) t -> p ci t", p=128)
                )

                # ---- GroupNorm stats: per-partition bn_stats, then group
                # aggregation and per-channel broadcast via tiny mask matmuls.
                stat3 = pg.tile([128, NCHUNK, 3], F32, tag="stat3")
                for ci in range(NCHUNK):
                    st6 = pg.tile([128, 2, 6], F32, tag="st6")
                    for half in range(2):
                        nc.vector.bn_stats(
                            out=st6[:, half, :],
                            in_=x_t[:, ci, half * 512:(half + 1) * 512],
                        )
                    nc.vector.bn_aggr(out=stat3[:, ci, 0:2], in_=st6)
                    nc.vector.tensor_mul(
                        stat3[:, ci, 2:3], stat3[:, ci, 0:1], stat3[:, ci, 0:1]
                    )
                psum_g = pp_small.tile([G, 3], F32, tag="pssm")
                for ci in range(NCHUNK):
                    nc.tensor.matmul(
                        psum_g, mfw_sb[:, ci, :], stat3[:, ci, :],
                        start=(ci == 0), stop=(ci == NCHUNK - 1),
                    )
                gsb = pg.tile([G, 3], F32, tag="gsb")
                nc.scalar.activation(
                    out=gsb, in_=psum_g, func=mybir.ActivationFunctionType.Copy,
                )
                gs = pg.tile([G, 2], F32, tag="gs")
                t0 = pg.tile([G, 1], F32, tag="t0")
                nc.vector.tensor_mul(t0, gsb[:, 0:1], gsb[:, 0:1])
                nc.vector.tensor_add(gs[:, 1:2], gsb[:, 1:2], gsb[:, 2:3])
                nc.vector.tensor_sub(gs[:, 1:2], gs[:, 1:2], t0)
                # rsqrt(v+eps) = exp(-0.5*ln(v+eps)); Ln/Exp share an ACT table set
                nc.scalar.activation(
                    out=gs[:, 1:2], in_=gs[:, 1:2],
                    func=mybir.ActivationFunctionType.Ln,
                    bias=eps_sb[0:G], scale=1.0,
                )
                nc.scalar.activation(
                    out=gs[:, 1:2], in_=gs[:, 1:2],
                    func=mybir.ActivationFunctionType.Exp,
                    bias=0.0, scale=-0.5,
                )
                nc.vector.tensor_copy(out=gs[:, 0:1], in_=gsb[:, 0:1])

                # ---- normalize + affine -> h (bf16) ----
                h_t = ph.tile([128, NCHUNK, HW], BF16, tag="h")
                # group->channel broadcast via DMA gather (stride-0 inner dim)
                # instead of mask matmuls: saves PE work + PSUM contention
                bc_all = pg.tile([128, NCHUNK, 2], F32, tag="bc_all")
                for ci in range(NCHUNK):
                    sl = gs[4 * ci:4 * ci + 4, :]
                    src = bass.AP(tensor=sl.tensor, offset=sl.offset,
                                  ap=[list(sl.ap[0][:2]), [0, 32], [1, 2]])
                    nc.scalar.dma_start(out=bc_all[:, ci, :], in_=src)
                for ci in range(NCHUNK):
                    psum_bc = bc_all[:, ci, :]
                    A = pg.tile([128, 1], F32, tag="A")
                    Bt = pg.tile([128, 1], F32, tag="B")
                    nc.vector.tensor_mul(A, psum_bc[:, 1:2], gamma_sb[:, ci:ci + 1])
                    nc.vector.tensor_mul(Bt, psum_bc[:, 0:1], A)
                    nc.vector.tensor_sub(Bt, beta_sb[:, ci:ci + 1], Bt)
                    nc.vector.tensor_scalar(
                        out=h_t[:, ci, :], in0=x_t[:, ci, :],
                        scalar1=A, scalar2=Bt,
                        op0=mybir.AluOpType.mult, op1=mybir.AluOpType.add,
                    )

                # ---- g (fused: wm^T h == q with k-side folded) ----
                g_t = pq.tile([128, NCHUNK, HW], BF16, tag="g")
                for mo in range(NCHUNK):
                    for ni in range(NNI):
                        ps = pp_big.tile([128, 512], F32, tag="ps")
                        for ci in range(NCHUNK):
                            nc.tensor.matmul(
                                ps,
                                wm_sb[:, ci, mo * 128:(mo + 1) * 128],
                                h_t[:, ci, ni * 512:(ni + 1) * 512],
                                start=(ci == 0), stop=(ci == NCHUNK - 1),
                            )
                        if fused:
                            nc.scalar.activation(
                                out=g_t[:, mo, ni * 512:(ni + 1) * 512], in_=ps,
                                func=mybir.ActivationFunctionType.Copy,
                            )
                        else:
                            nc.scalar.activation(
                                out=g_t[:, mo, ni * 512:(ni + 1) * 512], in_=ps,
                                func=mybir.ActivationFunctionType.Identity,
                                bias=bq_sb[:, mo:mo + 1], scale=1.0,
                            )

                if not fused:
                    k_t = pk.tile([128, NCHUNK, HW], BF16, tag="k")
                    for mo in range(NCHUNK):
                        for ni in range(NNI):
                            ps = pp_big.tile([128, 512], F32, tag="ps")
                            for ci in range(NCHUNK):
                                nc.tensor.matmul(
                                    ps,
                                    wk_sb[:, ci, mo * 128:(mo + 1) * 128],
                                    h_t[:, ci, ni * 512:(ni + 1) * 512],
                                    start=(ci == 0), stop=(ci == NCHUNK - 1),
                                )
                            nc.scalar.activation(
                                out=k_t[:, mo, ni * 512:(ni + 1) * 512], in_=ps,
                                func=mybir.ActivationFunctionType.Identity,
                                bias=bk_sb[:, mo:mo + 1], scale=1.0,
                            )
                    s_lhs = k_t
                else:
                    s_lhs = h_t

                # ---- vpT = (Wp Wv h)^T : [j, o'] ----
                vpT_t = pvt.tile([128, NJT, C], BF16, tag="vpT")
                for jo in range(NJT):
                    ps = pp_big.tile([128, 512], F32, tag="ps")
                    for ci in range(NCHUNK):
                        nc.tensor.matmul(
                            ps,
                            h_t[:, ci, jo * 128:(jo + 1) * 128],
                            wvp_sb[:, ci, :],
                            start=(ci == 0), stop=(ci == NCHUNK - 1),
                        )
                    nc.scalar.activation(
                        out=vpT_t[:, jo, :], in_=ps,
                        func=mybir.ActivationFunctionType.Copy,
                    )

                # ---- S^T = s_lhs^T g, E = exp, Z column sums (PE) ----
                e_t = pe.tile([128, NJT, HW], BF16, tag="e")
                psz = pp_z.tile([64, 512], F32, tag="psz")
                for jo in range(NJT):
                    for ni in range(NNI):
                        ps = pp_big.tile([128, 512], F32, tag="ps")
                        for ci in range(NCHUNK):
                            nc.tensor.matmul(
                                ps,
                                s_lhs[:, ci, jo * 128:(jo + 1) * 128],
                                g_t[:, ci, ni * 512:(ni + 1) * 512],
                                start=(ci == 0), stop=(ci == NCHUNK - 1),
                            )
                        nc.scalar.activation(
                            out=e_t[:, jo, ni * 512:(ni + 1) * 512], in_=ps,
                            func=mybir.ActivationFunctionType.Exp,
                        )
                        nc.tensor.matmul(
                            psz[ni * 32:ni * 32 + 1, :], ones_sb,
                            e_t[:, jo, ni * 512:(ni + 1) * 512],
                            start=(jo == 0), stop=(jo == NJT - 1),
                        )

                # ---- 1/Z, broadcast across partitions via rank-1 matmul ----
                zr = pzr.tile([1, HW], F32, tag="zr", name=f"zr_{rep}_{s}")
                for ni in range(NNI):
                    nc.vector.reciprocal(
                        out=zr[:, ni * 512:(ni + 1) * 512],
                        in_=psz[ni * 32:ni * 32 + 1, :]
                    )
                zrb = pzrb.tile([128, HW], F32, tag="zrb", name=f"zrb_{rep}_{s}")
                for ni in range(NNI):
                    ps = pp_big.tile([128, 512], F32, tag="ps", name=f"zb_{rep}_{s}_{ni}")
                    nc.tensor.matmul(
                        ps, onesf_sb, zr[:, ni * 512:(ni + 1) * 512],
                        start=True, stop=True,
                    )
                    nc.scalar.activation(
                        out=zrb[:, ni * 512:(ni + 1) * 512], in_=ps,
                        func=mybir.ActivationFunctionType.Copy,
                    )

                # ---- p = vpT^T E; tail y = p*zr + (x + bpp) ----
                for mo in range(NCHUNK):
                    y_t = py.tile([128, HW], F32, tag="y")
                    for ni in range(NNI):
                        ps = pp_big.tile([128, 512], F32, tag="ps")
                        for jo in range(NJT):
                            nc.tensor.matmul(
                                ps,
                                vpT_t[:, jo, mo * 128:(mo + 1) * 128],
                                e_t[:, jo, ni * 512:(ni + 1) * 512],
                                start=(jo == 0), stop=(jo == NJT - 1),
                            )
                        tmp = ptail.tile([128, 512], F32, tag="tmp")
                        nc.vector.tensor_mul(tmp, ps, zrb[:, ni * 512:(ni + 1) * 512])
                        nc.vector.scalar_tensor_tensor(
                            out=y_t[:, ni * 512:(ni + 1) * 512],
                            in0=x_t[:, mo, ni * 512:(ni + 1) * 512],
                            scalar=bpp_sb[:, mo:mo + 1],
                            in1=tmp,
                            op0=mybir.AluOpType.add, op1=mybir.AluOpType.add,
                        )
                    nc.gpsimd.dma_start(
                        out=y_ext[s, mo * 128:(mo + 1) * 128, :], in_=y_t
                    )

    _split_multiwait_drains(nc)
    return nc


def make_host_inputs(x, gn_gamma, gn_beta, Wq, bq, Wk, bk, Wv, bv, Wp, bp, fused):
    scale = 1.0 / math.sqrt(C)
    wvp_l = ((Wp.astype(np.float64) @ Wv.astype(np.float64)).T).astype(ml_dtypes.bfloat16)
    bpp = (Wp.astype(np.float64) @ bv.astype(np.float64) + bp).astype(np.float32)
    if fused:
        # wm = Wq^T Wk / sqrt(c): S^T = (h^T wm^T) ... exact when bq=bk=0
        wm_l = (Wq.T.astype(np.float64) @ Wk.astype(np.float64) * scale).astype(ml_dtypes.bfloat16)
    else:
        wm_l = np.ascontiguousarray(Wq.T * scale).astype(ml_dtypes.bfloat16)
    wk_l = np.ascontiguousarray(Wk.T).astype(ml_dtypes.bfloat16)
    vecs = np.stack([
        gn_gamma, gn_beta, bpp, bq * scale, bk
    ]).astype(np.float32)

    grp = np.arange(C) // (C // G)
    mfw = np.zeros((128, NCHUNK, G), np.float32)
    mbw = np.zeros((G, NCHUNK, 128), np.float32)
    for ci in range(NCHUNK):
        for p in range(128):
            g = grp[ci * 128 + p]
            mfw[p, ci, g] = 1.0 / (C // G)
            mbw[g, ci, p] = 1.0

    xr = np.ascontiguousarray(x.reshape(B, C, HW)).astype(np.float32)
    in_maps = []
    for i in range(N_CORES):
        m = {
            "x": xr[i * SPC:(i + 1) * SPC],
            "wm": wm_l, "wvp": wvp_l,
            "vecs": vecs, "mask_fwd": mfw, "mask_bwd": mbw,
        }
        if not fused:
            m["wk"] = wk_l
        in_maps.append(m)
    return in_maps


_nc_cache = {}


def kernel(x, gn_gamma, gn_beta, Wq, bq, Wk, bk, Wv, bv, Wp, bp):
    x = np.asarray(x, dtype=np.float32)
    args = {k: np.asarray(v, dtype=np.float32) for k, v in dict(
        gn_gamma=gn_gamma, gn_beta=gn_beta, Wq=Wq, bq=bq, Wk=Wk, bk=bk,
        Wv=Wv, bv=bv, Wp=Wp, bp=bp).items()}
    b, c, h, w = x.shape
    assert (b, c, h * w) == (B, C, HW), f"unexpected shape {x.shape}"

    # q/k fusion is exact only for zero q/k biases (the spec fill)
    fused = (np.abs(args["bq"]).max() == 0.0 and np.abs(args["bk"]).max() == 0.0)

    if fused not in _nc_cache:
        _nc_cache[fused] = build(reps=1, fused=fused)
    nc = _nc_cache[fused]

    in_maps = make_host_inputs(x, fused=fused, **args)
    res = run_bass_kernel_spmd(nc, in_maps, list(range(N_CORES)))
    y = np.concatenate([r["y"] for r in res.results], axis=0)
    return y.reshape(b, c, h, w).astype(np.float32)


if __name__ == "__main__":
    rng = np.random.default_rng(0)
    scale = 1.0 / math.sqrt(C)
    demo = dict(
        x=rng.standard_normal((B, C, H, W), dtype=np.float32),
        gn_gamma=np.ones(C, np.float32), gn_beta=np.zeros(C, np.float32),
        Wq=(rng.standard_normal((C, C)) * scale).astype(np.float32),
        bq=np.zeros(C, np.float32),
        Wk=(rng.standard_normal((C, C)) * scale).astype(np.float32),
        bk=np.zeros(C, np.float32),
        Wv=(rng.standard_normal((C, C)) * scale).astype(np.float32),
        bv=np.zeros(C, np.float32),
        Wp=(rng.standard_normal((C, C)) * scale).astype(np.float32),
        bp=np.zeros(C, np.float32),
    )
    out = kernel(**demo)
    print("kernel output:", out.shape, out.dtype, float(np.abs(out).max()))
